# revision 9
# baseline (speedup 1.0000x reference)
"""Trainium2 Bass kernel for nn_AudioImaginationForGLUE (fp8 pipeline).

Pure data-parallel across 8 NeuronCores: each core handles 4 samples.
Spans processed as two sequential phases (span 1 may read rows written
by span 0).

Math transformations (same as validated baseline):
  - audio-MLP second layer folded into K/V projections
  - key bias dropped (softmax shift invariance)
  - value bias folded into output-proj bias
  - attention scale folded into wq/bq
  - ragged span via indirect-DMA gather/scatter + write-mask merge

Precision/layout scheme:
  - audio pre-transposed AND pre-quantized to fp8e4 on the host; the
    staging transposes disappear entirely (aiT is a straight DMA).
  - the three 1024-token GEMMs (audio-MLP h1, K, V) run as fp8
    DoubleRow matmuls (2 MACs/cell/cycle). Weights are host-prescaled
    (x32 / x64) into fp8's normal range; descales fold into the
    eviction activations / the exp scale.
  - scores are computed TRANSPOSED directly (lhsT = kc token tile,
    rhs = block-diagonal qT): no softmax max-subtraction, no PE
    transposes; exp((kc.T q)/512 - 4*ln2) folds every descale plus an
    fp8-headroom bias into one activation; the bias cancels in the
    sum-normalization.
  - V carries a ones-column per head so the attention row-sums fall out
    of the ctx matmul itself (col DH); ctx runs DoubleRow over token
    pairs.
  - everything else (q projection, o-proj, LN, FFN, gates) runs bf16.

Scheduling:
  - span 0 gathers read hs_in directly (no dependency on the full
    hidden-state copy, which runs in the background), and their DMAs are
    emitted before every bulk weight DMA (the sync engine issues
    dma_starts in order, so emission order is DMA priority).
  - gather/scatter/merge/transpose-to-natural all operate on SAMPLE
    PAIRS (2x64 = 128 rows) to halve instruction counts and DMA trips.
  - span s+1's gathers are emitted inside span s's stage-B per-pair loop
    right after that pair's scatter (per-sample row ranges are
    disjoint); the spanT build + q projection are emitted after span
    s+1's first h1/V block so the PE crosses the span boundary without
    idling. Stage-A tiles live in the persistent pool so pool-close
    dependencies never gate the overlap.
  - the gates' span-side partial sum is computed before LN2 (it does not
    depend on o2), filling the LN stats pipeline stall.
  - stage-B weights + the hs copy DMAs drip between span-0 samples so
    they never head-of-line block latency-critical loads.
"""

import numpy as np
import ml_dtypes

import concourse.bass as bass
import concourse.mybir as mybir
import concourse.tile as tile
from concourse import bacc
from concourse.masks import make_identity
from concourse.bass_utils import run_bass_kernel_spmd

F32 = mybir.dt.float32
BF16 = mybir.dt.bfloat16
F8 = mybir.dt.float8e4
I32 = mybir.dt.int32
AF = mybir.ActivationFunctionType
AX = mybir.AxisListType
OP = mybir.AluOpType
DR = mybir.MatmulPerfMode.DoubleRow
NPF8 = ml_dtypes.float8_e4m3
NPBF = ml_dtypes.bfloat16

P = 128
B, S, H, NH, FF, A, TA, NSPAN, MAXL = 32, 512, 768, 12, 3072, 768, 1024, 2, 64
DH = H // NH          # 64
VW = 68               # v per-head stride: DH cols + ones col + pad (16B align)
HC = H // P           # 6 hidden chunks
HCP = HC // 2         # 3 chunk pairs (DoubleRow)
FC = FF // P          # 24 ffn chunks
TT = TA // P          # 8 audio token tiles
NCORES = 8
BPC = B // NCORES     # 4 samples per core
TBLK = 512
NBLK = TA // TBLK
NB = BPC * MAXL       # 256
SCALE = 1.0 / float(np.sqrt(DH))

SW1 = 32.0            # host prescale on mlp_w1 (fp8 range)
SKV = 64.0            # host prescale on wk_eff / wv_eff
SQ = 8.0              # qT eviction scale
EXP_SCALE = 1.0 / (SKV * SQ)
EXP_BIAS = float(-4.0 * np.log(2.0))  # fp8 headroom; cancels in the sum norm


def build_program():
    nc = bacc.Bacc("TRN2", target_bir_lowering=False, debug=False)

    t = {}
    t["hs_in"] = nc.dram_tensor("hs_in", [BPC * S, H], F32, kind="ExternalInput")
    t["audio"] = nc.dram_tensor("audio", [NSPAN, BPC, P, HC, TA], F8,
                                kind="ExternalInput")
    for nm in ("w_mw1", "w_wk", "w_wv"):
        t[nm] = nc.dram_tensor(nm, [P, HCP, 2, H], F8, kind="ExternalInput")
    for nm in ("w_wq", "w_wo", "w_gaw", "w_gtw"):
        t[nm] = nc.dram_tensor(nm, [H, H], BF16, kind="ExternalInput")
    t["w_fw1"] = nc.dram_tensor("w_fw1", [H, FF], BF16, kind="ExternalInput")
    t["w_fw2"] = nc.dram_tensor("w_fw2", [FF, H], BF16, kind="ExternalInput")
    for nm in ("p_mb1", "p_bq", "p_fb2", "p_gb", "p_g1", "p_b1", "p_g2", "p_b2"):
        t[nm] = nc.dram_tensor(nm, [P, HC], F32, kind="ExternalInput")
    t["p_fb1"] = nc.dram_tensor("p_fb1", [P, FC], F32, kind="ExternalInput")
    t["bo_row"] = nc.dram_tensor("bo_row", [1, H], BF16, kind="ExternalInput")
    t["ones_r"] = nc.dram_tensor("ones_r", [1, NB], BF16, kind="ExternalInput")
    t["meta"] = nc.dram_tensor("meta", [NSPAN, BPC // 2, 2 * MAXL, 3], I32,
                               kind="ExternalInput")
    t["hs_out"] = nc.dram_tensor("hs_out", [BPC * S, H], F32, kind="ExternalOutput")

    with tile.TileContext(nc) as tc, \
            nc.allow_low_precision("fp8/bf16 kernel; rel tolerance 2e-2"):
        _emit(nc, tc, t)
    nc.finalize()
    return nc


def _emit(nc, tc, t):
    hs_in, hs_out = t["hs_in"], t["hs_out"]

    with (
        tc.tile_pool(name="const", bufs=1) as cpool,
        tc.tile_pool(name="resw", bufs=1) as resw,
        tc.tile_pool(name="perbs", bufs=1) as perbs,
    ):
        gnat_t = [None] * (BPC // 2)
        wm_t = [None] * (BPC // 2)
        w1m_t = [None] * (BPC // 2)
        gi_t = [None] * (BPC // 2)

        # ---- span 0 fetches absolutely first: the gathers are on the
        # critical path to the first PE work ----
        for i in range(BPC // 2):
            _span_fetch(nc, t, 0, i, perbs, hs_in, gnat_t, wm_t, gi_t, w1m_t)

        # ---- constants ----
        ident = cpool.tile([P, P], F32, tag="ident")
        make_identity(nc, ident)
        identb = cpool.tile([P, P], BF16, tag="identb")
        make_identity(nc, identb)
        ones_col = cpool.tile([P, 1], BF16, tag="ones_col")
        nc.vector.memset(ones_col[:], 1.0)
        expb = cpool.tile([P, 1], F32, tag="expb")
        nc.vector.memset(expb[:], EXP_BIAS)
        ones_row = cpool.tile([1, NB], BF16, tag="ones_row")
        nc.sync.dma_start(out=ones_row[:], in_=t["ones_r"][:, :])
        eps_t = cpool.tile([P, 1], F32, tag="eps_t")
        nc.vector.memset(eps_t[:], 1e-5)

        packs = {}
        for nm in ("p_mb1", "p_bq", "p_fb1", "p_fb2", "p_gb",
                   "p_g1", "p_b1", "p_g2", "p_b2"):
            nch = FC if nm == "p_fb1" else HC
            pk = cpool.tile([P, nch], F32, tag=nm)
            nc.sync.dma_start(out=pk[:], in_=t[nm][:, :])
            packs[nm] = pk
        borow = cpool.tile([1, H], BF16, tag="borow")
        nc.sync.dma_start(out=borow[:], in_=t["bo_row"][:, :])

        # critical-path weights: wq (q proj), then the fp8 audio weights,
        # then the first audio block
        wres = {}
        for nm in ("w_wq",):
            ws = resw.tile([P, HC, H], BF16, tag=nm)
            nc.sync.dma_start(
                out=ws[:], in_=t[nm][:, :].rearrange("(c p) n -> p c n", p=P))
            wres[nm[2:]] = ws
        for nm in ("w_mw1", "w_wk", "w_wv"):      # fp8 DoubleRow layout
            ws = resw.tile([P, HCP, 2, H], F8, tag=nm)
            nc.sync.dma_start(out=ws[:], in_=t[nm][:, :, :, :])
            wres[nm[2:]] = ws
        ai_first = _stage_ai(nc, t, 0, 0, perbs)

        # stage-B weights + hs copy are background work: deferred into the span-0
        # stage-A loop (emitted between samples) so they never head-of-line
        # block the latency-critical small DMAs above.
        rows = BPC * S
        step = rows // 8
        bg_dmas = []
        for nm in ("w_wo", "w_gaw", "w_gtw"):
            def _w_load(nm=nm):
                ws = resw.tile([P, HC, H], BF16, tag=nm)
                nc.sync.dma_start(
                    out=ws[:],
                    in_=t[nm][:, :].rearrange("(c p) n -> p c n", p=P))
                wres[nm[2:]] = ws
            bg_dmas.append(_w_load)
        for i in range(8):
            def _copy(i=i):
                nc.sync.dma_start(out=hs_out[i * step:(i + 1) * step, :],
                                  in_=hs_in[i * step:(i + 1) * step, :])
            bg_dmas.append(_copy)

        spanT_cur = perbs.tile([P, HC, BPC, MAXL], BF16, tag="spanT", bufs=2)
        h1_pre = None
        for s in range(NSPAN):
            ctxT = perbs.tile([P, HC, BPC, MAXL], BF16, tag="ctxT", bufs=2)

            with tc.tile_pool(name=f"sA{s}", bufs=1) as pa:
                qT = None
                if s == 0:
                    with tc.tile_pool(name="ph0", bufs=1, space="PSUM") as qh:
                        for i in range(BPC // 2):
                            _span_build(nc, i, qh, ("tp", 2), ident, gnat_t,
                                        spanT_cur)
                        qT = _qproj(nc, perbs, qh, wres, packs, spanT_cur)
                ai_cur = ai_first if s == 0 else ai_nextspan
                with tc.tile_pool(name=f"psA{s}", bufs=1, space="PSUM") as qa:
                    for b in range(BPC):
                        ai_next = (_stage_ai(nc, t, s, b + 1, perbs)
                                   if b + 1 < BPC else None)
                        h1T, v = _stage_audio(nc, s, b, perbs, qa, wres,
                                              packs, ai_cur,
                                              h1_pre if (s > 0 and b == 0)
                                              else None)
                        if s > 0 and b == 0:
                            # span s's gathers completed during span s-1's
                            # stage-B; build spanT + qT while h1/V still
                            # stream through the PE
                            for ii in range(BPC // 2):
                                _span_build(nc, ii, qa, ("sc", 2), ident,
                                            gnat_t, spanT_cur)
                            qT = _qproj(nc, perbs, qa, wres, packs,
                                        spanT_cur)
                        _stage_attn(nc, s, b, perbs, qa, wres, identb, qT,
                                    ctxT, h1T, v, expb)
                        # drip the background DMAs between samples
                        if s == 0:
                            for k in range(3):
                                if bg_dmas:
                                    bg_dmas.pop(0)()
                        ai_cur = ai_next
                    while bg_dmas:
                        bg_dmas.pop(0)()

            if s + 1 < NSPAN:
                spanT_next = perbs.tile([P, HC, BPC, MAXL], BF16, tag="spanT",
                                        bufs=2)
                ai_nextspan = _stage_ai(nc, t, s + 1, 0, perbs)
                h1_pre = perbs.tile([P, HC, TA], F8, tag="h1T")

                def h1_cb(blk, qb_, ai=ai_nextspan, h1_pre=h1_pre):
                    off = blk * TBLK
                    for co in range(HC):
                        ph = qb_.tile([P, TBLK], F32, tag="st2", bufs=2)
                        for j in range(HCP):
                            nc.tensor.matmul(
                                ph[:, :],
                                wres["mw1"][:, j, :, co * P:(co + 1) * P],
                                ai[:, 2 * j:2 * j + 2, off:off + TBLK],
                                start=(j == 0), stop=(j == HCP - 1),
                                perf_mode=DR)
                        nc.scalar.activation(h1_pre[:, co, off:off + TBLK],
                                             ph[:, :], AF.Relu,
                                             bias=packs["p_mb1"][:, co:co + 1],
                                             scale=1.0 / SW1)

                def fetch_cb(i, s=s):
                    _span_fetch(nc, t, s + 1, i, perbs, hs_out,
                                gnat_t, wm_t, gi_t, w1m_t)
            else:
                spanT_next, fetch_cb, h1_cb, h1_pre = None, None, None, None

            with (
                tc.tile_pool(name=f"sB{s}", bufs=1) as pb,
                tc.tile_pool(name=f"psB{s}", bufs=1, space="PSUM") as qb,
            ):
                _stage_b(nc, t, s, pb, qb, wres, packs, identb, ones_col,
                         ones_row, eps_t, borow, spanT_cur, ctxT,
                         gnat_t, wm_t, w1m_t, gi_t, hs_out, fetch_cb, h1_cb)
            spanT_cur = spanT_next


def _stage_ai(nc, t, s, b, perbs):
    """DMA one sample's pre-transposed fp8 audio block."""
    aiT = perbs.tile([P, HC, TA], F8, tag="aiT", bufs=3)
    nc.sync.dma_start(out=aiT[:], in_=t["audio"][s, b, :, :, :])
    return aiT


def _span_fetch(nc, t, s, i, perbs, src, gnat_t, wm_t, gi_t, w1m_t):
    """Gather one sample PAIR's span rows (2x64=128) from `src`.

    No valid-mask multiply: pad rows hold real (clipped) hs values, every
    downstream op is per-token, and the write-mask excludes pad rows at
    scatter time, so masking them to zero is unnecessary.
    """
    meta = perbs.tile([2 * MAXL, 3], I32, tag="meta", bufs=4)
    nc.sync.dma_start(out=meta[:], in_=t["meta"][s, i, :, :])
    gi = meta[:, 0:1]
    gnat = perbs.tile([2 * MAXL, H], F32, tag="gnat", bufs=4)
    nc.gpsimd.indirect_dma_start(
        out=gnat[:], out_offset=None, in_=src[:, :],
        in_offset=bass.IndirectOffsetOnAxis(ap=gi, axis=0))
    gnat_t[i], wm_t[i], gi_t[i] = gnat, meta[:, 2:3].bitcast(F32), gi
    w1m_t[i] = meta[:, 1:2].bitcast(F32)


def _span_build(nc, i, qpool, qtag, ident, gnat_t, spanT):
    """PE transposes: one gathered pair (128 rows) -> spanT[:, :, 2i:2i+2, :]."""
    gnat = gnat_t[i]
    for c in range(0, HC, 2):
        pt = qpool.tile([P, 2, P], F32, tag=qtag[0], bufs=qtag[1])
        for j in range(2):
            nc.tensor.transpose(out=pt[:, j, :],
                                in_=gnat[:, (c + j) * P:(c + j + 1) * P],
                                identity=ident[:, :])
        nc.scalar.copy(spanT[:, c:c + 2, 2 * i:2 * i + 2, :], pt[:, :, :])


def _qproj(nc, pa, qh, wres, packs, spanT):
    """Batched q projection into block-diagonal fp8 layout."""
    qT = pa.tile([P, HC, BPC, 2, MAXL], F8, tag="qT", bufs=1)
    for co in range(HC):
        pq = qh.tile([P, NB], F32, tag="mm", bufs=3)
        for ci in range(HC):
            nc.tensor.matmul(pq[:, :], wres["wq"][:, ci, co * P:(co + 1) * P],
                             spanT[:, ci, :, :],
                             start=(ci == 0), stop=(ci == HC - 1))
        nc.scalar.activation(qT[0:DH, co, :, 0, :], pq[0:DH, :], AF.Identity,
                             bias=packs["p_bq"][0:DH, co:co + 1], scale=SQ)
        nc.scalar.activation(qT[DH:P, co, :, 1, :], pq[DH:P, :], AF.Identity,
                             bias=packs["p_bq"][DH:P, co:co + 1], scale=SQ)
        nc.vector.tensor_scalar_mul(qT[0:DH, co, :, 1, :], pq[0:DH, :], 0.0)
        nc.vector.tensor_scalar_mul(qT[DH:P, co, :, 0, :], pq[DH:P, :], 0.0)
    return qT


def _stage_audio(nc, s, b, pa, qa, wres, packs, aiT, h1_pre=None):
    """h1 and V for one sample (fp8 DoubleRow); no hidden-state deps."""

    if h1_pre is not None:
        h1T = h1_pre            # computed during the previous stage-B
    else:
        # ---- h1 = relu((ai @ mw1*32)/32 + mb1) -> fp8 ----
        h1T = pa.tile([P, HC, TA], F8, tag="h1T")
        for blk in range(NBLK):
            off = blk * TBLK
            for co in range(HC):
                ph = qa.tile([P, TBLK], F32, tag="mm", bufs=3)
                for j in range(HCP):
                    nc.tensor.matmul(ph[:, :],
                                     wres["mw1"][:, j, :, co * P:(co + 1) * P],
                                     aiT[:, 2 * j:2 * j + 2, off:off + TBLK],
                                     start=(j == 0), stop=(j == HCP - 1),
                                     perf_mode=DR)
                nc.scalar.activation(h1T[:, co, off:off + TBLK], ph[:, :],
                                     AF.Relu,
                                     bias=packs["p_mb1"][:, co:co + 1],
                                     scale=1.0 / SW1)

    # ---- v = (h1 @ wv*64)/64 -> fp8 [128(t), TT, NH, VW]; ones col at DH ----
    v = pa.tile([P, TT, NH, VW], F8, tag="v")
    nc.vector.memset(v[:, :, :, DH:DH + 1], 1.0)
    for tt in range(TT):
        p1 = qa.tile([P, 512], F32, tag="mm", bufs=3)
        p2 = qa.tile([P, 512], F32, tag="mm", bufs=3)
        for j in range(HCP):
            lhs = h1T[:, 2 * j:2 * j + 2, tt * P:(tt + 1) * P]
            nc.tensor.matmul(p1[:, :], lhs, wres["wv"][:, j, :, 0:512],
                             start=(j == 0), stop=(j == HCP - 1), perf_mode=DR)
            nc.tensor.matmul(p2[:, :256], lhs, wres["wv"][:, j, :, 512:768],
                             start=(j == 0), stop=(j == HCP - 1), perf_mode=DR)
        nc.vector.tensor_scalar_mul(
            v[:, tt, 0:8, 0:DH],
            p1[:, :].rearrange("p (h d) -> p h d", h=8), 1.0 / SKV)
        nc.vector.tensor_scalar_mul(
            v[:, tt, 8:12, 0:DH],
            p2[:, :256].rearrange("p (h d) -> p h d", h=4), 1.0 / SKV)
    return h1T, v


def _stage_attn(nc, s, b, pa, qa, wres, identb, qT, ctxT, h1T, v, expb):
    """kc + attention for one sample."""
    # ---- kc (one head-pair ahead) + attention ----
    def make_kc(hp):
        kc = pa.tile([P, TA], F8, tag="kc", bufs=2)
        for nh in range(2):
            pk = qa.tile([P, 512], F32, tag="mm", bufs=3)
            for j in range(HCP):
                nc.tensor.matmul(pk[:, :],
                                 wres["wk"][:, j, :, hp * P:(hp + 1) * P],
                                 h1T[:, 2 * j:2 * j + 2,
                                     nh * 512:(nh + 1) * 512],
                                 start=(j == 0), stop=(j == HCP - 1),
                                 perf_mode=DR)
            if nh == 0:
                nc.vector.tensor_copy(kc[:, 0:512], pk[:, :])
            else:
                nc.scalar.copy(kc[:, 512:1024], pk[:, :])
        return kc

    ctx_nat = pa.tile([MAXL, H], BF16, tag="ctx_nat", bufs=1)
    kc_cur = make_kc(0)
    for hp in range(NH // 2):
        # scores transposed: sT[t, (hh,l)] = sum_d kc[d,t] * qT[d,(hh,l)]
        attT = pa.tile([P, TT, P], F8, tag="attT", bufs=2)
        for g in range(2):
            sT = qa.tile([P, 4, P], F32, tag="sc", bufs=2)
            for ti in range(4):
                tt = g * 4 + ti
                nc.tensor.matmul(sT[:, ti, :],
                                 kc_cur[:, tt * P:(tt + 1) * P],
                                 qT[:, hp, b, :, :],
                                 start=True, stop=True)
            nc.scalar.activation(attT[:, g * 4:(g + 1) * 4, :], sT[:, :, :],
                                 AF.Exp, bias=expb[:, :1], scale=EXP_SCALE)
        if hp + 1 < NH // 2:
            kc_cur = make_kc(hp + 1)
        # ctx (+ row-sums in col DH): [MAXL, DH+1] per head, DR over t-pairs
        for hh in range(2):
            h = 2 * hp + hh
            pc = qa.tile([MAXL, DH + 1], F32, tag="pc", bufs=2)
            for t2 in range(TT // 2):
                nc.tensor.matmul(pc[:, :],
                                 attT[:, 2 * t2:2 * t2 + 2,
                                      hh * DH:(hh + 1) * DH],
                                 v[:, 2 * t2:2 * t2 + 2, h, 0:DH + 1],
                                 start=(t2 == 0), stop=(t2 == TT // 2 - 1),
                                 perf_mode=DR)
            rec = pa.tile([MAXL, 1], F32, tag="rec", bufs=2)
            nc.vector.reciprocal(rec[:], pc[:, DH:DH + 1])
            nc.vector.tensor_scalar_mul(ctx_nat[:, h * DH:(h + 1) * DH],
                                        pc[:, 0:DH], rec[:, :1])

    # ---- transpose ctx -> ctxT[:, :, b, :] (bf16, single-pass) ----
    for c in range(0, HC, 2):
        pt = qa.tile([P, 8, P], BF16, tag="sc", bufs=2)
        for j in range(2):
            nc.tensor.transpose(out=pt[:, j, :MAXL],
                                in_=ctx_nat[:, (c + j) * P:(c + j + 1) * P],
                                identity=identb[:MAXL, :MAXL])
        nc.scalar.copy(ctxT[:, c:c + 2, b, :], pt[:, :2, :MAXL])


def _layernorm_T(nc, qb, pb, xT, outT, gpack, bpack, ones_col, ones_row,
                 eps_t):
    """LayerNorm over the feature (partition-chunk) axis, bf16 pipeline."""
    psum = qb.tile([1, NB], F32, tag="st", bufs=2)
    for c in range(HC):
        nc.tensor.matmul(psum[:, :], ones_col[:, :], xT[:, c, :, :],
                         start=(c == 0), stop=(c == HC - 1))
    m_row = pb.tile([1, NB], BF16, tag="m_row", bufs=1)
    nc.vector.tensor_scalar_mul(m_row[:], psum[:, :], 1.0 / H)

    sq = pb.tile([P, HC, NB], BF16, tag="sq", bufs=1)
    for c in range(HC):
        nc.scalar.activation(sq[:, c, :], xT[:, c, :, :], AF.Square)
    psq = qb.tile([1, NB], F32, tag="st", bufs=2)
    for c in range(HC):
        nc.tensor.matmul(psq[:, :], ones_col[:, :], sq[:, c, :],
                         start=(c == 0), stop=(c == HC - 1))
    var = pb.tile([1, NB], BF16, tag="var", bufs=1)
    msq = pb.tile([1, NB], BF16, tag="msq", bufs=1)
    nc.scalar.activation(msq[:], m_row[:], AF.Square)
    nc.vector.tensor_scalar(out=var[:], in0=psq[:, :], scalar1=1.0 / H,
                            scalar2=None, op0=OP.mult)
    nc.vector.tensor_tensor(out=var[:], in0=var[:], in1=msq[:],
                            op=OP.subtract)
    pm_b = qb.tile([P, NB], F32, tag="st2", bufs=2)
    nc.tensor.matmul(pm_b[:, :], ones_row[:1, :P], m_row[:1, :],
                     start=True, stop=True)
    pv_b = qb.tile([P, NB], F32, tag="st2", bufs=2)
    nc.tensor.matmul(pv_b[:, :], ones_row[:1, :P], var[:1, :],
                     start=True, stop=True)
    rstd = pb.tile([P, NB], F32, tag="rstd", bufs=1)
    nc.scalar.activation(rstd[:], pv_b[:, :], AF.Sqrt, bias=eps_t[:, :1])
    nc.vector.reciprocal(rstd[:], rstd[:])
    for c in range(HC):
        nc.vector.tensor_tensor(out=outT[:, c, :, :], in0=xT[:, c, :, :],
                                in1=pm_b[:, :], op=OP.subtract)
        nc.vector.tensor_tensor(out=outT[:, c, :, :], in0=outT[:, c, :, :],
                                in1=rstd[:, :], op=OP.mult)
        nc.vector.tensor_scalar(out=outT[:, c, :, :], in0=outT[:, c, :, :],
                                scalar1=gpack[:, c:c + 1],
                                scalar2=bpack[:, c:c + 1],
                                op0=OP.mult, op1=OP.add)


def _stage_b(nc, t, s, pb, qb, wres, packs, ident, ones_col, ones_row, eps_t,
             borow, spanT, ctxT, gnat_t, wm_t, w1m_t, gi_t, hs_out, fetch_cb,
             h1_cb):
    """Batched fusion tail (bf16): o-proj, LN1, FFN, LN2, gates, merge."""

    # ---- x1 = ctx @ wo + bo + span ----
    x1 = pb.tile([P, HC, BPC, MAXL], BF16, tag="xT", bufs=2)
    for co in range(HC):
        po = qb.tile([P, NB], F32, tag="mmB", bufs=3)
        for ci in range(HC):
            nc.tensor.matmul(po[:, :], wres["wo"][:, ci, co * P:(co + 1) * P],
                             ctxT[:, ci, :, :],
                             start=(ci == 0), stop=False)
        nc.tensor.matmul(po[:, :], borow[:1, co * P:(co + 1) * P],
                         ones_row[:1, :], start=False, stop=True)
        nc.vector.tensor_tensor(out=x1[:, co, :, :], in0=po[:, :],
                                in1=spanT[:, co, :, :], op=OP.add)

    # ---- LN1 (next span's first h1 block fills the stats stall) ----
    o1 = pb.tile([P, HC, BPC, MAXL], BF16, tag="out1T", bufs=1)
    _layernorm_T(nc, qb, pb, x1, o1, packs["p_g1"], packs["p_b1"],
                 ones_col, ones_row, eps_t)
    if h1_cb is not None:
        h1_cb(0, qb)

    # ---- FFN (bf16 streams) ----
    GRP = 3
    acc = pb.tile([P, HC, NB], F32, tag="acc", bufs=1)
    for sup in range(FC // GRP):
        cf0 = sup * GRP
        f1 = pb.tile([P, HC, GRP * P], BF16, tag="f1c", bufs=2)
        nc.sync.dma_start(
            out=f1[:], in_=t["w_fw1"][:, cf0 * P:(cf0 + GRP) * P]
            .rearrange("(c p) n -> p c n", p=P))
        f2t = pb.tile([P, GRP, H], BF16, tag="f2c", bufs=2)
        nc.sync.dma_start(
            out=f2t[:], in_=t["w_fw2"][cf0 * P:(cf0 + GRP) * P, :]
            .rearrange("(c p) n -> p c n", p=P))
        hf = pb.tile([P, GRP, NB], BF16, tag="hf", bufs=2)
        for jg in range(GRP):
            cf = cf0 + jg
            ph = qb.tile([P, NB], F32, tag="mmB", bufs=3)
            for ci in range(HC):
                nc.tensor.matmul(ph[:, :], f1[:, ci, jg * P:(jg + 1) * P],
                                 o1[:, ci, :, :],
                                 start=(ci == 0), stop=(ci == HC - 1))
            nc.scalar.activation(hf[:, jg, :], ph[:, :], AF.Gelu,
                                 bias=packs["p_fb1"][:, cf:cf + 1])
        for co in range(HC):
            pacc = qb.tile([P, NB], F32, tag="mmB", bufs=3)
            for jg in range(GRP):
                nc.tensor.matmul(pacc[:, :], f2t[:, jg, co * P:(co + 1) * P],
                                 hf[:, jg, :], start=(jg == 0),
                                 stop=(jg == GRP - 1))
            if sup == 0:
                nc.vector.tensor_copy(acc[:, co, :], pacc[:, :])
            else:
                nc.vector.tensor_tensor(out=acc[:, co, :], in0=acc[:, co, :],
                                        in1=pacc[:, :], op=OP.add)

    # x2 = acc + fb2 + o1
    x2 = pb.tile([P, HC, BPC, MAXL], BF16, tag="xT", bufs=2)
    for co in range(HC):
        nc.vector.tensor_scalar(out=x2[:, co, :, :], in0=acc[:, co, :],
                                scalar1=packs["p_fb2"][:, co:co + 1],
                                scalar2=None, op0=OP.add)
        nc.vector.tensor_tensor(out=x2[:, co, :, :], in0=x2[:, co, :, :],
                                in1=o1[:, co, :, :], op=OP.add)

    # ---- gates, span side (independent of LN2: fills its stats stall) ----
    gspan = pb.tile([P, HC, NB], BF16, tag="gspan", bufs=1)
    for co in range(HC):
        pgs = qb.tile([P, NB], F32, tag="mmB", bufs=3)
        for ci in range(HC):
            nc.tensor.matmul(pgs[:, :],
                             wres["gtw"][:, ci, co * P:(co + 1) * P],
                             spanT[:, ci, :, :],
                             start=(ci == 0), stop=(ci == HC - 1))
        nc.scalar.activation(gspan[:, co, :], pgs[:, :], AF.Identity,
                             bias=packs["p_gb"][:, co:co + 1])
    if h1_cb is not None:
        h1_cb(1, qb)

    # ---- LN2 ----
    o2 = pb.tile([P, HC, BPC, MAXL], BF16, tag="out2T", bufs=1)
    _layernorm_T(nc, qb, pb, x2, o2, packs["p_g2"], packs["p_b2"],
                 ones_col, ones_row, eps_t)

    # ---- gates, o2 side + combine + fused ----
    gate = pb.tile([P, HC, BPC, MAXL], BF16, tag="gateT", bufs=1)
    fused = pb.tile([P, HC, BPC, MAXL], BF16, tag="fusedT", bufs=1)
    for co in range(HC):
        pg = qb.tile([P, NB], F32, tag="mmB", bufs=3)
        for ci in range(HC):
            nc.tensor.matmul(pg[:, :], wres["gaw"][:, ci, co * P:(co + 1) * P],
                             o2[:, ci, :, :],
                             start=(ci == 0), stop=(ci == HC - 1))
        gtmp = pb.tile([P, NB], BF16, tag="gtmp", bufs=2)
        nc.vector.tensor_tensor(out=gtmp[:], in0=pg[:, :],
                                in1=gspan[:, co, :], op=OP.add)
        nc.scalar.activation(gate[:, co, :, :], gtmp[:, :], AF.Sigmoid)
        # fused = span + gate*(o2 - span), emitted per-chunk so the DVE
        # starts the merge tail while the PE is still on gates matmuls
        nc.vector.tensor_tensor(out=fused[:, co, :, :], in0=o2[:, co, :, :],
                                in1=spanT[:, co, :, :], op=OP.subtract)
        nc.vector.tensor_tensor(out=fused[:, co, :, :], in0=fused[:, co, :, :],
                                in1=gate[:, co, :, :], op=OP.mult)
        nc.vector.tensor_tensor(out=fused[:, co, :, :], in0=fused[:, co, :, :],
                                in1=spanT[:, co, :, :], op=OP.add)

    # ---- per sample-pair: back to natural, merge, scatter, prep next ----
    for i in range(BPC // 2):
        fnat = pb.tile([2 * MAXL, H], F32, tag="fnat", bufs=2)
        for c in range(0, HC, 2):
            pt = qb.tile([2 * MAXL, 4, P], BF16, tag="mmB", bufs=3)
            for j in range(2):
                nc.tensor.transpose(out=pt[:, j, :],
                                    in_=fused[:, c + j, 2 * i:2 * i + 2, :],
                                    identity=ident[:, :])
            nc.scalar.copy(fnat[:, c * P:(c + 2) * P], pt[:, :2, :])
        # merged = wm*fnat + (1-wm)*gnat  (wm==0 rows stay exactly gnat)
        merged = pb.tile([2 * MAXL, H], F32, tag="merged", bufs=2)
        nc.vector.tensor_scalar_mul(merged[:], fnat[:], wm_t[i])
        nc.vector.scalar_tensor_tensor(out=merged[:], in0=gnat_t[i][:],
                                       scalar=w1m_t[i], in1=merged[:],
                                       op0=OP.mult, op1=OP.add)
        nc.gpsimd.indirect_dma_start(
            out=hs_out[:, :],
            out_offset=bass.IndirectOffsetOnAxis(ap=gi_t[i], axis=0),
            in_=merged[:], in_offset=None)
        if fetch_cb is not None:
            fetch_cb(i)


# ============================ host glue ============================

_NC_CACHE = None


def _get_program():
    global _NC_CACHE
    if _NC_CACHE is None:
        _NC_CACHE = build_program()
    return _NC_CACHE


def _pack_dr_fp8(w, scale):
    """[H, H] -> DoubleRow lhsT layout [P, HCP, 2, H] in fp8e4."""
    a = np.asarray(w, np.float64) * scale
    a = a.reshape(HCP, 2, P, H).transpose(2, 0, 1, 3)
    a = np.clip(a, -240.0, 240.0)
    return np.ascontiguousarray(a.astype(np.float32)).astype(NPF8)


def _fold_weights(inp):
    f64 = lambda x: np.asarray(x, np.float64)
    w = {}
    w["w_mw1"] = _pack_dr_fp8(inp["mlp_w1"], SW1)
    w["w_wk"] = _pack_dr_fp8(f64(inp["mlp_w2"]) @ f64(inp["wk"]), SKV)
    w["w_wv"] = _pack_dr_fp8(f64(inp["mlp_w2"]) @ f64(inp["wv"]), SKV)
    bv_eff = f64(inp["mlp_b2"]) @ f64(inp["wv"]) + f64(inp["bv"])
    bo_eff = (bv_eff @ f64(inp["wo"]) + f64(inp["bo"])).astype(np.float32)
    w["w_wq"] = (f64(inp["wq"]) * SCALE).astype(np.float32).astype(NPBF)
    bq_eff = (f64(inp["bq"]) * SCALE * SQ).astype(np.float32)
    w["w_wo"] = np.asarray(inp["wo"], np.float32).astype(NPBF)
    w["w_gaw"] = np.asarray(inp["ga_w"], np.float32).astype(NPBF)
    w["w_gtw"] = np.asarray(inp["gt_w"], np.float32).astype(NPBF)
    w["w_fw1"] = np.asarray(inp["ffn_w1"], np.float32).astype(NPBF)
    w["w_fw2"] = np.asarray(inp["ffn_w2"], np.float32).astype(NPBF)
    gb_eff = (f64(inp["ga_b"]) + f64(inp["gt_b"])).astype(np.float32)

    def pack(vec, nch):
        return np.ascontiguousarray(
            np.asarray(vec, np.float32).reshape(nch, P).T)

    w["p_mb1"] = pack(inp["mlp_b1"], HC)
    w["p_bq"] = pack(bq_eff, HC)
    w["p_fb1"] = pack(inp["ffn_b1"], FC)
    w["p_fb2"] = pack(inp["ffn_b2"], HC)
    w["p_gb"] = pack(gb_eff, HC)
    w["p_g1"] = pack(inp["ln1_g"], HC)
    w["p_b1"] = pack(inp["ln1_b"], HC)
    w["p_g2"] = pack(inp["ln2_g"], HC)
    w["p_b2"] = pack(inp["ln2_b"], HC)
    w["bo_row"] = bo_eff.reshape(1, H).astype(NPBF)
    w["ones_r"] = np.ones((1, NB), np.float32).astype(NPBF)
    return w


def _prep_audio(au):
    """[B, NSPAN, TA, A] f32 -> [B, NSPAN, P, HC, TA] fp8 (pre-transposed)."""
    at = au.transpose(0, 1, 3, 2)                       # [B, NSPAN, A, TA]
    at = at.reshape(B, NSPAN, HC, P, TA).transpose(0, 1, 3, 2, 4)
    return np.ascontiguousarray(at).astype(NPF8)


def _span_meta(spans, active, core):
    """Pack [gidx, vmsk, wmsk] int32 [NSPAN, BPC//2, 2*MAXL, 3] (b-pairs)."""
    ar = np.arange(MAXL)
    meta = np.zeros((NSPAN, BPC, MAXL, 3), np.int32)
    for s in range(NSPAN):
        for bl in range(BPC):
            bg = core * BPC + bl
            st = int(spans[bg, s, 0])
            en = min(int(spans[bg, s, 1]), S)
            L = max(en - st, 0)
            idx = np.clip(st + ar, 0, S - 1)
            meta[s, bl, :, 0] = bl * S + idx
            vm = (ar < L).astype(np.float32)
            wm = vm * np.float32(bool(active[bg, s]))
            meta[s, bl, :, 1] = (1.0 - wm).view(np.int32)
            meta[s, bl, :, 2] = wm.view(np.int32)
    return np.ascontiguousarray(meta.reshape(NSPAN, BPC // 2, 2 * MAXL, 3))


def _run(inputs, trace=False):
    nc = _get_program()
    hs = np.ascontiguousarray(inputs["hidden_states"], np.float32)
    au8 = _prep_audio(np.ascontiguousarray(inputs["audio_inputs"], np.float32))
    spans = np.asarray(inputs["spans_token_pos"])
    active = np.asarray(inputs["in_audios"])
    w = _fold_weights(inputs)

    in_maps = []
    for c in range(NCORES):
        m = dict(w)
        m["hs_in"] = hs[c * BPC:(c + 1) * BPC].reshape(BPC * S, H)
        m["audio"] = np.ascontiguousarray(
            au8[c * BPC:(c + 1) * BPC].transpose(1, 0, 2, 3, 4))
        m["meta"] = _span_meta(spans, active, c)
        in_maps.append(m)

    kw = {}
    if trace:
        kw = dict(trace=True, trace_cores=[0])
    res = run_bass_kernel_spmd(nc, in_maps, core_ids=list(range(NCORES)), **kw)
    out = np.empty((B, S, H), np.float32)
    for c in range(NCORES):
        out[c * BPC:(c + 1) * BPC] = res.results[c]["hs_out"].reshape(BPC, S, H)
    return out, res


def kernel(**inputs):
    out, _ = _run(inputs, trace=False)
    return out
